# revision 1
# baseline (speedup 1.0000x reference)
"""Trainium2 Bass kernel for nn_Decoder (NeRF-style 9-layer MLP, Softplus(beta=100)).

Strategy (pure data parallel over 8 cores, feature-major layout):
  - activations live in SBUF as z_l = 100 * y_l  (softplus "raw" outputs), shape
    [features<=100 partitions, points free-dim]; weights are stationary lhsT.
  - per layer: 4x fp32 matmul (N=512 each) into one PSUM tile [100, 2048],
    ACT: e = Exp(psum + 100*b)   (== e^u, +inf for u > 88.7 -- handled below)
    ACT: l = Ln(e + 1)           (== softplus(u) for moderate u)
    DVE: z = max(min(l, max(u, 40)), u), u = psum + 100*b   (single fused custom
         op; exact in fp32: for u<=40 picks l, above 40 softplus(u)==u in fp32 and
         the outer max discards Ln-table garbage (Ln breaks for inputs > 2.3e19))
  - skip connection (layer 4) handled by DMAing the raw input into partitions
    98:100 of the layer-3 output tile; layer-4 weights columns are scaled to match.
  - layer 8 (100->1, no activation): matmul + DVE bias add, DMA out.
The exp/ln pair lives in one ACT table set (natural_log_exp_and_others): no
table switching. All matmuls fp32 (exact). Supertiles are emitted
software-pipelined in groups of GRP=4 (layers interleaved across the group) so
each engine's in-order stream never head-of-line blocks on the serial
MM->Exp->Ln->fin chain of a single supertile.
"""

import numpy as np

import concourse.bass as bass
import concourse.tile as tile
from concourse import bacc, mybir
from concourse import bass_utils
from concourse.bass_interp import get_hw_module

F32 = mybir.dt.float32
F32R = mybir.dt.float32r
ACTF = mybir.ActivationFunctionType

N_CORES = 8
N_TOTAL = 1048576
P = N_TOTAL // N_CORES          # 131072 points per core
T = 2048                        # supertile (points per ACT instruction; 4 PSUM banks)
NT = P // T                     # supertiles per core
DIMS = [2, 100, 100, 100, 98, 100, 100, 100, 100, 1]

_SOFTPLUS_FIN = None


def _get_softplus_fin():
    """Register (once) the fused custom-DVE op:
        out = min(in0, max(in1 + s0, s1))
    in0 = Ln(e+1) tile (SBUF), in1 = psum (PSUM), s0 = +100*b [P,1], s1 = 85.0
    """
    global _SOFTPLUS_FIN
    if _SOFTPLUS_FIN is not None:
        return _SOFTPLUS_FIN
    from concourse import dve_ops
    from concourse.dve_spec import Spec, Src0, Src1, C0, C1, lower, maxx, minn, _has_src1
    from concourse.dve_uop import DveOpSpec
    from concourse.dve_table_gen import dve_ver_for

    name = "SOFTPLUS_FIN_ANT"
    # u = in1 + s0;  z = max(min(in0, max(u, s1)), u)
    # With s1=40: for u<=40 picks in0 (= Ln(e^u + 1), accurate there); for u>40
    # softplus(u) == u in fp32, and the outer max(_, u) also discards any
    # garbage the Ln table emits for huge inputs (x > 2.3e19) of either sign.
    _uu = Src1 + C0
    spec = Spec(
        body=maxx(minn(Src0, maxx(_uu, C1)), _uu),
        reference=lambda in0, in1, s0, s1, imm2: np.maximum(
            np.minimum(in0, np.maximum(in1.astype(np.float32) + s0, s1)),
            in1.astype(np.float32) + s0,
        ),
    )
    op = dve_ops.DveOp(name, spec, subdim=False, uops_sha={})
    dve_ops.OPS.append(op)
    dve_ops.CUSTOM_DVE_SPECS[name] = spec
    dve_ops._SUB_OPCODE_FOR_NAME[name] = (
        dve_ops._CUSTOM_DVE_ROW_BASE + len(dve_ops.OPS) - 1
    )
    assert dve_ops._SUB_OPCODE_FOR_NAME[name] < 0x20
    for ver in ("v3", "v4"):
        uops = lower(spec, ver=ver)
        tmp = DveOpSpec(
            name=name,
            opcode=dve_ops.get_dve_sub_opcode(name),
            uops=uops,
            rd1_en=_has_src1(spec),
        )
        op.uops_sha[ver] = tmp.sha(ver)
    _SOFTPLUS_FIN = op
    return op


_TABLES_PATCHED = False


def _patch_act_tables():
    """Make natural_log_exp_and_others the only table set advertising Exp/Ln,
    so the table-load placement pass keeps one set loaded for the whole kernel
    instead of thrashing between exp_and_others and natural_log (~1024 reloads,
    ~1.3 ms). Set positions are preserved (position == act_func_set_id)."""
    global _TABLES_PATCHED
    if _TABLES_PATCHED:
        return
    import concourse.hw_specs as hw_specs
    import concourse.bacc as bacc_mod

    orig = hw_specs.get_activation_tables
    EXP = ACTF.Exp
    LN = ACTF.Ln

    def patched(module_arch):
        tables = {k: set(v) for k, v in orig(module_arch).items()}
        for name, funcs in tables.items():
            if name != "natural_log_exp_and_others":
                funcs.discard(EXP)
                funcs.discard(LN)
        return tables

    hw_specs.get_activation_tables = patched
    bacc_mod.get_activation_tables = patched
    _TABLES_PATCHED = True


def _build_program(T=1024, psum_bufs=4, ebufs=8, lbufs=8, mbufs=10, use_f32r=False, GRP=4, xbufs=6, PAIR_LN=False):
    NT = P // T
    _patch_act_tables()
    sp_fin = _get_softplus_fin()
    MMDT = F32R if use_f32r else F32
    nc = bacc.Bacc(
        "TRN2",
        target_bir_lowering=False,
        debug=False,
        enable_asserts=False,
        num_devices=N_CORES,
    )

    # DRAM I/O (per core)
    xt_d = nc.dram_tensor("xt", [2, P], F32, kind="ExternalInput")
    lhsT_d = []
    bias_d = []
    for l in range(9):
        in_dim = 100 if l == 4 else DIMS[l]
        out_dim = DIMS[l + 1]
        lhsT_d.append(
            nc.dram_tensor(f"lhsT{l}", [in_dim, out_dim], F32, kind="ExternalInput")
        )
        if l < 8:
            bias_d.append(
                nc.dram_tensor(f"bias{l}", [out_dim, 1], F32, kind="ExternalInput")
            )
    b8_d = nc.dram_tensor("b8", [1, 1], F32, kind="ExternalInput")
    y_d = nc.dram_tensor("y", [1, P], F32, kind="ExternalOutput")

    with tile.TileContext(nc) as tc:
        with (
            tc.tile_pool(name="wpool", bufs=1) as wpool,
            tc.tile_pool(name="xpool", bufs=xbufs) as xpool,
            tc.tile_pool(name="psum", bufs=psum_bufs, space="PSUM") as pspool,
            tc.tile_pool(name="epool", bufs=ebufs) as epool,
            tc.tile_pool(name="lpool", bufs=lbufs) as lpool,
            tc.tile_pool(name="mpool", bufs=mbufs) as mpool,
            tc.tile_pool(name="opool", bufs=2) as opool,
        ):
            # --- preload weights/biases ---
            wts = []
            bts = []
            for l in range(9):
                in_dim = 100 if l == 4 else DIMS[l]
                out_dim = DIMS[l + 1]
                wt = wpool.tile([in_dim, out_dim], MMDT, tag=f"w{l}")
                nc.sync.dma_start(wt[:], lhsT_d[l].ap().bitcast(MMDT))
                wts.append(wt)
                if l < 8:
                    bt = wpool.tile([out_dim, 1], F32, tag=f"b{l}")
                    nc.sync.dma_start(bt[:], bias_d[l].ap())
                    bts.append(bt)
            b8t = wpool.tile([1, 1], F32, tag="b8")
            nc.sync.dma_start(b8t[:], b8_d.ap())

            # --- main loop: software-pipeline GRP supertiles layer-by-layer
            # (interleaved emission so each engine's in-order stream ping-pongs
            # between independent supertiles instead of head-of-line blocking
            # on the serial per-supertile chain) ---
            assert NT % GRP == 0

            def emit_l8(sls, prevs):
                for i in range(GRP):
                    ps = pspool.tile([100, T], F32, tag="ps")
                    for j in range(T // 512):
                        js = bass.ts(j, 512)
                        nc.tensor.matmul(
                            ps[0:1, js], wts[8][:], prevs[i][0:100, js],
                            start=True, stop=True,
                        )
                    out_t = opool.tile([1, T], F32, tag="out")
                    nc.vector.tensor_scalar_add(out_t[:], ps[0:1, :], b8t[0:1, 0:1])
                    nc.sync.dma_start(y_d.ap()[:, sls[i]], out_t[:])

            deferred = None
            for g in range(NT // GRP):
                ts_ids = [g * GRP + i for i in range(GRP)]
                sls = [bass.ts(t, T) for t in ts_ids]
                xts = []
                for sl in sls:
                    xt = xpool.tile([2, T], MMDT, tag="xt")
                    nc.sync.dma_start(xt[:], xt_d.ap()[:, sl].bitcast(MMDT))
                    xts.append(xt)
                prevs = list(xts)
                for l in range(8):
                    if l == 2 and deferred is not None:
                        emit_l8(*deferred)
                        deferred = None
                    in_dim = 100 if l == 4 else DIMS[l]
                    out_dim = DIMS[l + 1]
                    pss = []
                    for i in range(GRP):
                        ps = pspool.tile([100, T], F32, tag="ps")
                        for j in range(T // 512):
                            js = bass.ts(j, 512)
                            nc.tensor.matmul(
                                ps[0:out_dim, js],
                                wts[l][:],
                                prevs[i][0:in_dim, js],
                                start=True,
                                stop=True,
                            )
                        pss.append(ps)
                    # Exp per psum tile; pairs of Exp outputs share one
                    # [100, 2*T] e tile so each Ln instruction covers two
                    # supertiles (halves the Ln per-instruction overhead
                    # without a group-wide barrier).
                    es = []
                    if PAIR_LN:
                        for i in range(0, GRP, 2):
                            e2 = epool.tile([100, 2 * T], F32, tag="e")
                            for h in range(2):
                                nc.scalar.activation(
                                    e2[0:out_dim, bass.ts(h, T)],
                                    pss[i + h][0:out_dim, :], ACTF.Exp,
                                    bias=bts[l][:, 0:1], scale=1.0,
                                )
                            es.append(e2)
                        lts = []
                        for e2 in es:
                            lt = lpool.tile([100, 2 * T], F32, tag="l")
                            nc.scalar.activation(
                                lt[0:out_dim, :], e2[0:out_dim, :], ACTF.Ln, bias=1.0
                            )
                            lts.append(lt)
                        lslices = [
                            lts[i // 2][0:out_dim, bass.ts(i % 2, T)]
                            for i in range(GRP)
                        ]
                    else:
                        for i in range(GRP):
                            e = epool.tile([100, T], F32, tag="e")
                            nc.scalar.activation(
                                e[0:out_dim, :], pss[i][0:out_dim, :], ACTF.Exp,
                                bias=bts[l][:, 0:1], scale=1.0,
                            )
                            es.append(e)
                        lslices = []
                        for i in range(GRP):
                            lt = lpool.tile([100, T], F32, tag="l")
                            nc.scalar.activation(
                                lt[0:out_dim, :], es[i][0:out_dim, :], ACTF.Ln,
                                bias=1.0,
                            )
                            lslices.append(lt[0:out_dim, :])
                    nprevs = []
                    for i in range(GRP):
                        m = mpool.tile([100, T], MMDT, tag="m7" if l == 7 else "m")
                        if l == 3:
                            nc.sync.dma_start(
                                m[98:100, :], xt_d.ap()[:, sls[i]].bitcast(MMDT)
                            )
                        nc.vector._custom_dve(
                            sp_fin,
                            out=m[0:out_dim, :],
                            in0=lslices[i],
                            in1=pss[i][0:out_dim, :],
                            s0=bts[l][:, 0:1],
                            s1=40.0,
                        )
                        nprevs.append(m)
                    prevs = nprevs
                if deferred is not None:
                    emit_l8(*deferred)
                deferred = (sls, prevs)
            if deferred is not None:
                emit_l8(*deferred)

    nc.compile()
    nc.m = get_hw_module(nc.m)
    return nc


def _transform_weights(inputs):
    """Host-side weight/bias transform -> per-program DRAM tensors (shared
    across cores)."""
    W = [np.asarray(inputs[f"W{l}"], dtype=np.float32) for l in range(9)]
    b = [np.asarray(inputs[f"b{l}"], dtype=np.float32) for l in range(9)]
    t = {}
    t["lhsT0"] = np.ascontiguousarray((100.0 * W[0]).T)
    for l in (1, 2, 3, 5, 6, 7):
        t[f"lhsT{l}"] = np.ascontiguousarray(W[l].T)
    t["lhsT4"] = np.ascontiguousarray(
        np.concatenate([W[4][:, 2:].T, (100.0 * W[4][:, :2]).T], axis=0)
    )
    t["lhsT8"] = np.ascontiguousarray(W[8].T / 100.0)
    for l in range(8):
        t[f"bias{l}"] = np.ascontiguousarray((100.0 * b[l]).reshape(-1, 1))
    t["b8"] = np.ascontiguousarray(b[8].reshape(1, 1))
    return t


_NC_CACHE = None


def kernel(**inputs) -> np.ndarray:
    global _NC_CACHE
    if _NC_CACHE is None:
        _NC_CACHE = _build_program()
    nc = _NC_CACHE

    x = np.asarray(inputs["input"], dtype=np.float32)
    assert x.shape == (N_TOTAL, 2)
    shared = _transform_weights(inputs)

    in_maps = []
    for c in range(N_CORES):
        m = dict(shared)
        m["xt"] = np.ascontiguousarray(x[c * P : (c + 1) * P].T)
        in_maps.append(m)

    res = bass_utils.run_bass_kernel_spmd(nc, in_maps, core_ids=list(range(N_CORES)))
    y = np.concatenate([res.results[c]["y"][0] for c in range(N_CORES)])
    return y.reshape(N_TOTAL, 1).astype(np.float32)



# revision 10
# speedup vs baseline: 1.8857x; 1.8857x over previous
"""Trainium2 Bass kernel for nn_Decoder (NeRF-style 9-layer MLP, Softplus(beta=100)).

Strategy (pure data parallel over 8 cores, feature-major layout):
  - activations live in SBUF as z_l = 100 * y_l (scaled softplus outputs), shape
    [features<=100 partitions, points free-dim]; weights are stationary lhsT.
  - matmuls run in float32r (1 cycle/row for free-dim >= 256, 4x faster than
    plain fp32; numerically exact in fp32 accumulation).
  - per layer, per supertile [100, T]:
      PE : psum = W z                        (T/512 matmuls, N=512 each)
      ACT: s = Sigmoid(-psum - 100b)         (= sigma(-u), one table op)
      DVE: z' = max(u + g, g)                (one fused custom op)
            where u = psum + 100b,  p = min(s, 1-s),
                  g = (c2*p + c1)*p  ~=  -ln(1-p)  (minimax on [0, 1/2])
    Math: softplus(u) = relu(u) + (-ln(1 - min(sigma(-u), sigma(u)))).
    The deg-2 minimax of -ln(1-p) has max err 5.4e-3 in z units (5.4e-5 in y),
    end-to-end rel err ~7e-4. Exact in both tails (p->0 as |u|->inf).
  - skip connection (layer 4) handled by DMAing the raw input into partitions
    98:100 of the layer-3 output tile; layer-4 weight columns scaled to match.
  - layer 8 (100->1, no activation): matmul into psum row 0, DMA straight to
    DRAM; the scalar bias b8 is added on the host after the gather.
Supertiles are emitted software-pipelined in groups of GRP=4 (layers
interleaved across the group) so each engine's in-order stream never
head-of-line blocks on the serial MM->ACT->DVE chain of a single supertile.
"""

import numpy as np

import concourse.bass as bass
import concourse.tile as tile
from concourse import bacc, mybir
from concourse import bass_utils
from concourse.bass_interp import get_hw_module

F32 = mybir.dt.float32
F32R = mybir.dt.float32r
ACTF = mybir.ActivationFunctionType

N_CORES = 8
N_TOTAL = 1048576
P = N_TOTAL // N_CORES          # 131072 points per core
DIMS = [2, 100, 100, 100, 98, 100, 100, 100, 100, 1]

# deg-2 minimax of -ln(1-p) on [0, 0.5]: g = (SP_C2*p + SP_C1)*p
SP_C1 = 0.94427875
SP_C2 = 0.86235463

_SOFTPLUS_SIG = None


def _get_softplus_sig():
    """Register (once) the fused custom-DVE op computing, per element,
        z = max(u + g, g)   with u = in1 + s0,  p = min(in0, 1-in0),
                                 g = (imm2*p + s1)*p
    in0 = sigma(-u) tile (SBUF), in1 = psum (PSUM), s0 = +100*b [P,1],
    s1 = SP_C1, imm2 = SP_C2.  Exactly 8 ALU stages (v3 pipeline depth)."""
    global _SOFTPLUS_SIG
    if _SOFTPLUS_SIG is not None:
        return _SOFTPLUS_SIG
    from concourse import dve_ops
    from concourse.dve_spec import (
        Spec, Src0, Src1, C0, C1, C2, One, lower, maxx, minn, _has_src1,
    )
    from concourse.dve_uop import DveOpSpec

    name = "SOFTPLUS_SIG_ANT"
    _p = minn(Src0, One - Src0)
    _u = Src1 + C0
    _g = (C2 * _p + C1) * _p
    body = maxx(_u + _g, _g)

    def _ref(in0, in1, s0, s1, imm2):
        s = np.asarray(in0, dtype=np.float32)
        u = (np.asarray(in1, dtype=np.float32) + np.asarray(s0, np.float32)).astype(
            np.float32
        )
        w = (np.float32(1.0) - s).astype(np.float32)
        p = np.minimum(s, w)
        g = (
            (np.float32(imm2) * p + np.asarray(s1, np.float32)).astype(np.float32) * p
        ).astype(np.float32)
        return np.maximum((u + g).astype(np.float32), g)

    spec = Spec(body=body, reference=_ref)
    op = dve_ops.DveOp(name, spec, subdim=False, uops_sha={})
    dve_ops.OPS.append(op)
    dve_ops.CUSTOM_DVE_SPECS[name] = spec
    dve_ops._SUB_OPCODE_FOR_NAME[name] = (
        dve_ops._CUSTOM_DVE_ROW_BASE + len(dve_ops.OPS) - 1
    )
    assert dve_ops._SUB_OPCODE_FOR_NAME[name] < 0x20
    for ver in ("v3", "v4"):
        uops = lower(spec, ver=ver)
        tmp = DveOpSpec(
            name=name,
            opcode=dve_ops.get_dve_sub_opcode(name),
            uops=uops,
            rd1_en=_has_src1(spec),
        )
        op.uops_sha[ver] = tmp.sha(ver)
    _SOFTPLUS_SIG = op
    return _SOFTPLUS_SIG


def _build_program(T=1024, psum_bufs=4, sbufs=8, mbufs=10, GRP=4, xbufs=6):
    NT = P // T
    sp_sig = _get_softplus_sig()
    nc = bacc.Bacc(
        "TRN2",
        target_bir_lowering=False,
        debug=False,
        enable_asserts=False,
        num_devices=N_CORES,
    )

    # DRAM I/O (per core)
    xt_d = nc.dram_tensor("xt", [2, P], F32, kind="ExternalInput")
    lhsT_d = []
    bneg_d = []
    bpos_d = []
    for l in range(8):
        in_dim = 100 if l == 4 else DIMS[l]
        out_dim = DIMS[l + 1]
        lhsT_d.append(
            nc.dram_tensor(f"lhsT{l}", [in_dim, out_dim], F32, kind="ExternalInput")
        )
        bneg_d.append(
            nc.dram_tensor(f"bneg{l}", [out_dim, 1], F32, kind="ExternalInput")
        )
        bpos_d.append(
            nc.dram_tensor(f"bpos{l}", [out_dim, 1], F32, kind="ExternalInput")
        )
    # layer 8 weights, one [100, GRP] tensor per group slot: column i holds
    # W8/100, the rest zeros -- so GRP supertiles' outputs accumulate into
    # distinct partitions of one [GRP, T] psum tile.
    w8_d = [
        nc.dram_tensor(f"lhsT8v{i}", [100, 4], F32, kind="ExternalInput")
        for i in range(4)
    ]
    y_d = nc.dram_tensor("y", [1, P], F32, kind="ExternalOutput")

    with tile.TileContext(nc) as tc:
        with (
            tc.tile_pool(name="wpool", bufs=1) as wpool,
            tc.tile_pool(name="xpool", bufs=xbufs) as xpool,
            tc.tile_pool(name="psum", bufs=psum_bufs, space="PSUM") as pspool,
            tc.tile_pool(name="spool", bufs=sbufs) as spool,
            tc.tile_pool(name="mpool", bufs=mbufs) as mpool,
            tc.tile_pool(name="opool", bufs=4) as opool,
        ):
            # --- preload weights/biases ---
            wts = []
            bnegs = []
            bposs = []
            for l in range(8):
                in_dim = 100 if l == 4 else DIMS[l]
                out_dim = DIMS[l + 1]
                wt = wpool.tile([in_dim, out_dim], F32R, tag=f"w{l}")
                nc.sync.dma_start(wt[:], lhsT_d[l].ap().bitcast(F32R))
                wts.append(wt)
                bn = wpool.tile([out_dim, 1], F32, tag=f"bn{l}")
                nc.sync.dma_start(bn[:], bneg_d[l].ap())
                bnegs.append(bn)
                bp = wpool.tile([out_dim, 1], F32, tag=f"bp{l}")
                nc.sync.dma_start(bp[:], bpos_d[l].ap())
                bposs.append(bp)
            w8ts = []
            for i in range(4):
                w8t = wpool.tile([100, 4], F32R, tag=f"w8v{i}")
                nc.sync.dma_start(w8t[:], w8_d[i].ap().bitcast(F32R))
                w8ts.append(w8t)

            # --- main loop ---
            assert NT % GRP == 0 and GRP == 4

            def emit_l8(sls, prevs):
                # accumulate GRP supertiles' scalar outputs into distinct
                # partitions of one [4, T] psum region via one-hot weights
                ps = pspool.tile([100, T], F32, tag="ps")
                for i in range(GRP):
                    for j in range(T // 512):
                        js = bass.ts(j, 512)
                        nc.tensor.matmul(
                            ps[0:4, js], w8ts[i][:], prevs[i][0:100, js],
                            start=(i == 0), stop=(i == GRP - 1),
                        )
                out_t = opool.tile([4, T], F32, tag="out")
                nc.scalar.copy(out_t[:], ps[0:4, :])
                for i in range(GRP):
                    nc.sync.dma_start(y_d.ap()[:, sls[i]], out_t[i : i + 1, :])

            deferred = None
            for g in range(NT // GRP):
                ts_ids = [g * GRP + i for i in range(GRP)]
                sls = [bass.ts(t, T) for t in ts_ids]
                xts = []
                for sl in sls:
                    xt = xpool.tile([2, T], F32R, tag="xt")
                    nc.sync.dma_start(xt[:], xt_d.ap()[:, sl].bitcast(F32R))
                    xts.append(xt)
                prevs = list(xts)
                for l in range(8):
                    if l == 2 and deferred is not None:
                        emit_l8(*deferred)
                        deferred = None
                    in_dim = 100 if l == 4 else DIMS[l]
                    out_dim = DIMS[l + 1]
                    pss = []
                    for i in range(GRP):
                        ps = pspool.tile([100, T], F32, tag="ps")
                        for j in range(T // 512):
                            js = bass.ts(j, 512)
                            nc.tensor.matmul(
                                ps[0:out_dim, js],
                                wts[l][:],
                                prevs[i][0:in_dim, js],
                                start=True,
                                stop=True,
                            )
                        pss.append(ps)
                    sts = []
                    for i in range(GRP):
                        st = spool.tile([100, T], F32, tag="s")
                        nc.scalar.activation(
                            st[0:out_dim, :], pss[i][0:out_dim, :], ACTF.Sigmoid,
                            bias=bnegs[l][:, 0:1], scale=-1.0,
                        )
                        sts.append(st)
                    nprevs = []
                    for i in range(GRP):
                        m = mpool.tile([100, T], F32R, tag="m7" if l == 7 else "m")
                        if l == 3:
                            nc.sync.dma_start(
                                m[98:100, :], xt_d.ap()[:, sls[i]].bitcast(F32R)
                            )
                        nc.vector._custom_dve(
                            sp_sig,
                            out=m[0:out_dim, :],
                            in0=sts[i][0:out_dim, :],
                            in1=pss[i][0:out_dim, :],
                            s0=bposs[l][:, 0:1],
                            s1=SP_C1,
                            imm2=SP_C2,
                        )
                        nprevs.append(m)
                    prevs = nprevs
                if deferred is not None:
                    emit_l8(*deferred)
                deferred = (sls, prevs)
            if deferred is not None:
                emit_l8(*deferred)

    nc.compile()
    nc.m = get_hw_module(nc.m)
    return nc


def _transform_weights(inputs):
    """Host-side weight/bias transform -> per-program DRAM tensors (shared
    across cores)."""
    W = [np.asarray(inputs[f"W{l}"], dtype=np.float32) for l in range(9)]
    b = [np.asarray(inputs[f"b{l}"], dtype=np.float32) for l in range(9)]
    t = {}
    t["lhsT0"] = np.ascontiguousarray((100.0 * W[0]).T)
    for l in (1, 2, 3, 5, 6, 7):
        t[f"lhsT{l}"] = np.ascontiguousarray(W[l].T)
    t["lhsT4"] = np.ascontiguousarray(
        np.concatenate([W[4][:, 2:].T, (100.0 * W[4][:, :2]).T], axis=0)
    )
    for i in range(4):
        v = np.zeros((100, 4), dtype=np.float32)
        v[:, i] = W[8].reshape(-1) / 100.0
        t[f"lhsT8v{i}"] = v
    for l in range(8):
        t[f"bneg{l}"] = np.ascontiguousarray((-100.0 * b[l]).reshape(-1, 1))
        t[f"bpos{l}"] = np.ascontiguousarray((100.0 * b[l]).reshape(-1, 1))
    return t


_NC_CACHE = None


def kernel(**inputs) -> np.ndarray:
    global _NC_CACHE
    if _NC_CACHE is None:
        _NC_CACHE = _build_program()
    nc = _NC_CACHE

    x = np.asarray(inputs["input"], dtype=np.float32)
    assert x.shape == (N_TOTAL, 2)
    shared = _transform_weights(inputs)

    in_maps = []
    for c in range(N_CORES):
        m = dict(shared)
        m["xt"] = np.ascontiguousarray(x[c * P : (c + 1) * P].T)
        in_maps.append(m)

    res = bass_utils.run_bass_kernel_spmd(nc, in_maps, core_ids=list(range(N_CORES)))
    b8 = np.float32(np.asarray(inputs["b8"], dtype=np.float32).reshape(()))
    y = np.concatenate([res.results[c]["y"][0] for c in range(N_CORES)])
    y = (y + b8).astype(np.float32)
    return y.reshape(N_TOTAL, 1)


# revision 38
# speedup vs baseline: 1.9410x; 1.0293x over previous
"""Trainium2 Bass kernel for nn_Decoder (NeRF-style 9-layer MLP, Softplus(beta=100)).

Strategy (pure data parallel over 8 cores, feature-major layout):
  - activations live in SBUF as z_l = 100 * y_l (scaled softplus outputs), shape
    [features<=100 partitions, points free-dim]; weights are stationary lhsT.
  - matmuls run in float32r (1 cycle/row for free-dim >= 256, 4x faster than
    plain fp32; numerically exact in fp32 accumulation).
  - per layer, per supertile [100, T]:
      PE : psum = W z                        (T/512 matmuls, N=512 each)
      ACT: s = Sigmoid(-psum - 100b)         (= sigma(-u), one table op)
      DVE: z' = max(u + g, g)                (one fused custom op)
            where u = psum + 100b,  p = min(s, 1-s),
                  g = (c2*p + c1)*p  ~=  -ln(1-p)  (minimax on [0, 1/2])
    Math: softplus(u) = relu(u) + (-ln(1 - min(sigma(-u), sigma(u)))).
    The deg-2 minimax of -ln(1-p) has max err 5.4e-3 in z units (5.4e-5 in y),
    end-to-end rel err ~7e-4. Exact in both tails (p->0 as |u|->inf).
  - skip connection (layer 4) handled by DMAing the raw input into partitions
    98:100 of the layer-3 output tile; layer-4 weight columns scaled to match.
  - layer 8 (100->1, no activation): matmul into psum row 0, DMA straight to
    DRAM; the scalar bias b8 is added on the host after the gather.
Supertiles are emitted software-pipelined in groups of GRP=4 (layers
interleaved across the group) so each engine's in-order stream never
head-of-line blocks on the serial MM->ACT->DVE chain of a single supertile.
"""

import numpy as np

import concourse.bass as bass
import concourse.tile as tile
from concourse import bacc, mybir
from concourse import bass_utils
from concourse.bass_interp import get_hw_module

F32 = mybir.dt.float32
F32R = mybir.dt.float32r
ACTF = mybir.ActivationFunctionType

N_CORES = 8
N_TOTAL = 1048576
P = N_TOTAL // N_CORES          # 131072 points per core
DIMS = [2, 100, 100, 100, 98, 100, 100, 100, 100, 1]

# deg-2 minimax of -ln(1-p) on [0, 0.5]: g = (SP_C2*p + SP_C1)*p
SP_C1 = 0.94427875
SP_C2 = 0.86235463

_SOFTPLUS_SIG = None


def _wpack_layout():
    """Column layout of the packed weight/bias tensor [100, WPK].  Layer 0's
    weights and biases occupy the first columns so a slim head DMA can launch
    the pipeline while the rest streams in."""
    wcol = [0] * 8
    bncol = [0] * 8
    bpcol = [0] * 8
    c = DIMS[1]          # lhsT0
    bncol[0] = c
    bpcol[0] = c + 1
    c += 2
    head = c             # end of the slim head
    for l in range(1, 8):
        wcol[l] = c
        c += DIMS[l + 1]
    w8col = []
    for i in range(16):
        w8col.append(c)
        c += 16
    for l in range(1, 8):
        bncol[l] = c
        bpcol[l] = c + 1
        c += 2
    return wcol, w8col, bncol, bpcol, head, c


def _get_softplus_sig():
    """Register (once) the fused custom-DVE op computing, per element,
        z = max(u + g, g)   with u = in1 + s0,  p = min(in0, 1-in0),
                                 g = (imm2*p + s1)*p
    in0 = sigma(-u) tile (SBUF), in1 = psum (PSUM), s0 = +100*b [P,1],
    s1 = SP_C1, imm2 = SP_C2.  Exactly 8 ALU stages (v3 pipeline depth)."""
    global _SOFTPLUS_SIG
    if _SOFTPLUS_SIG is not None:
        return _SOFTPLUS_SIG
    from concourse import dve_ops
    from concourse.dve_spec import (
        Spec, Src0, Src1, C0, C1, C2, One, lower, maxx, minn, _has_src1,
    )
    from concourse.dve_uop import DveOpSpec

    name = "SOFTPLUS_SIG_ANT"
    _p = minn(Src0, One - Src0)
    _u = Src1 + C0
    _g = (C2 * _p + C1) * _p
    body = maxx(_u + _g, _g)

    def _ref(in0, in1, s0, s1, imm2):
        s = np.asarray(in0, dtype=np.float32)
        u = (np.asarray(in1, dtype=np.float32) + np.asarray(s0, np.float32)).astype(
            np.float32
        )
        w = (np.float32(1.0) - s).astype(np.float32)
        p = np.minimum(s, w)
        g = (
            (np.float32(imm2) * p + np.asarray(s1, np.float32)).astype(np.float32) * p
        ).astype(np.float32)
        return np.maximum((u + g).astype(np.float32), g)

    spec = Spec(body=body, reference=_ref)
    op = dve_ops.DveOp(name, spec, subdim=False, uops_sha={})
    dve_ops.OPS.append(op)
    dve_ops.CUSTOM_DVE_SPECS[name] = spec
    dve_ops._SUB_OPCODE_FOR_NAME[name] = (
        dve_ops._CUSTOM_DVE_ROW_BASE + len(dve_ops.OPS) - 1
    )
    assert dve_ops._SUB_OPCODE_FOR_NAME[name] < 0x20
    for ver in ("v3", "v4"):
        uops = lower(spec, ver=ver)
        tmp = DveOpSpec(
            name=name,
            opcode=dve_ops.get_dve_sub_opcode(name),
            uops=uops,
            rd1_en=_has_src1(spec),
        )
        op.uops_sha[ver] = tmp.sha(ver)
    _SOFTPLUS_SIG = op
    return _SOFTPLUS_SIG


def _build_program(T=1024, psum_bufs=4, sbufs=8, mbufs=16, GRP=4, xbufs=6):
    NT = P // T
    sp_sig = _get_softplus_sig()
    nc = bacc.Bacc(
        "TRN2",
        target_bir_lowering=False,
        debug=False,
        enable_asserts=False,
        num_devices=N_CORES,
    )

    # DRAM I/O (per core).  All weights/biases arrive in ONE packed [100, WPK]
    # tensor (one DMA instead of ~30 -- the serial HWDGE preamble was 20us):
    # columns [wcol[l] : wcol[l]+out_dim] hold lhsT_l (valid on partitions
    # 0:in_dim), then 4 one-hot layer-8 weight blocks of 4 columns each, then
    # 8 columns of bneg and 8 of bpos (valid on partitions 0:out_dim).
    wcol, w8col, bncol, bpcol, WHEAD, WPK = _wpack_layout()
    xt_d = nc.dram_tensor("xt", [2, P], F32, kind="ExternalInput")
    wpk_d = nc.dram_tensor("wpack", [100, WPK], F32, kind="ExternalInput")
    y_d = nc.dram_tensor("y", [1, P], F32, kind="ExternalOutput")

    with tile.TileContext(nc) as tc:
        with (
            tc.tile_pool(name="wpool", bufs=1) as wpool,
            tc.tile_pool(name="xpool", bufs=xbufs) as xpool,
            tc.tile_pool(name="psum", bufs=psum_bufs, space="PSUM") as pspool,
            tc.tile_pool(name="spool", bufs=sbufs) as spool,
            tc.tile_pool(name="mpool", bufs=mbufs) as mpool,
            tc.tile_pool(name="opool", bufs=4) as opool,
        ):
            # dummy 1-element sigmoid with no upstream deps: walrus places
            # the ACT table load before it, so the load runs at t~0 instead
            # of gating the first real sigmoid
            dz = wpool.tile([1, 1], F32, tag="dz")
            nc.vector.memset(dz[:], 0.0)
            dzo = wpool.tile([1, 1], F32, tag="dzo")
            nc.scalar.activation(dzo[:], dz[:], ACTF.Sigmoid, bias=0.0, scale=1.0)
            # --- first input tile, then weights: slim head (layer-0 weights
            # and biases) first so the first matmul chain launches early
            xt0 = xpool.tile([2, T], F32R, tag="xt")
            nc.sync.dma_start(xt0[:], xt_d.ap()[:, bass.ts(0, T)].bitcast(F32R))
            wpk = wpool.tile([100, WPK], F32R, tag="wpack")
            nc.sync.dma_start(wpk[0:100, 0:WHEAD], wpk_d.ap()[:, 0:WHEAD].bitcast(F32R))
            nc.sync.dma_start(wpk[0:100, WHEAD:WPK], wpk_d.ap()[:, WHEAD:WPK].bitcast(F32R))
            wts = []
            for l in range(8):
                in_dim = 100 if l == 4 else DIMS[l]
                out_dim = DIMS[l + 1]
                wts.append(wpk[0:in_dim, wcol[l] : wcol[l] + out_dim])
            w8ts = [wpk[0:100, w8col[i] : w8col[i] + 16] for i in range(16)]
            bnegs = [
                wpk[0 : DIMS[l + 1], bncol[l] : bncol[l] + 1].bitcast(F32)
                for l in range(8)
            ]
            bposs = [
                wpk[0 : DIMS[l + 1], bpcol[l] : bpcol[l] + 1].bitcast(F32)
                for l in range(8)
            ]

            # --- main loop ---
            assert NT % GRP == 0 and GRP == 4

            def emit_l8(col0, prevs):
                # accumulate the GRP supertiles' scalar outputs into a single
                # [16, 256] psum tile (row 4i+j = supertile i, quarter j) via
                # one-hot weight columns; the ACT eviction copy then has free
                # size 256 so it barely perturbs the sigma stream.
                nq = T // 256
                ps = pspool.tile([100, T], F32, tag="ps")
                k = 0
                for i in range(GRP):
                    for j in range(nq):
                        js = bass.ts(j, 256)
                        nc.tensor.matmul(
                            ps[0:16, 0:256], w8ts[k], prevs[i][0:100, js],
                            start=(k == 0), stop=(k == GRP * nq - 1),
                        )
                        k += 1
                out_t = opool.tile([16, 256], F32, tag="out")
                nc.scalar.copy(out_t[:], ps[0:16, 0:256])
                # one DMA for the whole group: row 4i+j of out_t lands at
                # y[col0 + 1024*i + 256*j : +256] -- rows map contiguously
                nc.sync.dma_start(
                    y_d.ap()[:, col0 : col0 + GRP * T], out_t[0:16, :]
                )

            deferred = None
            for g in range(NT // GRP):
                ts_ids = [g * GRP + i for i in range(GRP)]
                sls = [bass.ts(t, T) for t in ts_ids]
                xts = []
                for i, sl in enumerate(sls):
                    if g == 0 and i == 0:
                        xts.append(xt0)
                        continue
                    xt = xpool.tile([2, T], F32R, tag="xt")
                    nc.sync.dma_start(xt[:], xt_d.ap()[:, sl].bitcast(F32R))
                    xts.append(xt)
                prevs = list(xts)
                for l in range(8):
                    if l == 2 and deferred is not None:
                        emit_l8(*deferred)
                        deferred = None
                    in_dim = 100 if l == 4 else DIMS[l]
                    out_dim = DIMS[l + 1]
                    pss = []
                    for i in range(GRP):
                        ps = pspool.tile([100, T], F32, tag="ps")
                        for j in range(T // 512):
                            js = bass.ts(j, 512)
                            rhs = prevs[i][0:in_dim, js]
                            nc.tensor.matmul(
                                ps[0:out_dim, js],
                                wts[l],
                                rhs,
                                start=True,
                                stop=True,
                            )
                        pss.append(ps)
                    sts = []
                    for i in range(GRP):
                        st = spool.tile([100, T], F32, tag="s")
                        nc.scalar.activation(
                            st[0:out_dim, :], pss[i][0:out_dim, :], ACTF.Sigmoid,
                            bias=bnegs[l], scale=-1.0,
                        )
                        sts.append(st)
                    nprevs = []
                    for i in range(GRP):
                        m = mpool.tile([100, T], F32R, tag="m7" if l == 7 else "m")
                        out_ap = m[0:out_dim, :]
                        if l == 3:
                            nc.sync.dma_start(
                                m[98:100, :], xt_d.ap()[:, sls[i]].bitcast(F32R)
                            )
                        nc.vector._custom_dve(
                            sp_sig,
                            out=out_ap,
                            in0=sts[i][0:out_dim, :],
                            in1=pss[i][0:out_dim, :],
                            s0=bposs[l],
                            s1=SP_C1,
                            imm2=SP_C2,
                        )
                        nprevs.append(m)
                    prevs = nprevs
                if deferred is not None:
                    emit_l8(*deferred)
                deferred = (ts_ids[0] * T, prevs)
            if deferred is not None:
                emit_l8(*deferred)

    nc.compile()
    nc.m = get_hw_module(nc.m)
    return nc


def _transform_weights(inputs):
    """Host-side weight/bias transform -> one packed DRAM tensor (shared
    across cores)."""
    W = [np.asarray(inputs[f"W{l}"], dtype=np.float32) for l in range(9)]
    b = [np.asarray(inputs[f"b{l}"], dtype=np.float32) for l in range(9)]
    wcol, w8col, bncol, bpcol, WHEAD, WPK = _wpack_layout()
    pk = np.zeros((100, WPK), dtype=np.float32)
    lhsT = {}
    lhsT[0] = (100.0 * W[0]).T.astype(np.float32)
    for l in (1, 2, 3, 5, 6, 7):
        lhsT[l] = W[l].T
    # layer-4 input tile layout: partitions 0:98 = z3, 98:100 = raw x
    lhsT[4] = np.concatenate(
        [W[4][:, 2:].T, (100.0 * W[4][:, :2]).T.astype(np.float32)], axis=0
    )
    for l in range(8):
        t = lhsT[l]
        pk[0 : t.shape[0], wcol[l] : wcol[l] + t.shape[1]] = t
    for i in range(16):
        pk[:, w8col[i] + i] = W[8].reshape(-1) / 100.0
    for l in range(8):
        bn = (-100.0 * b[l]).astype(np.float32)
        bp = (100.0 * b[l]).astype(np.float32)
        pk[0 : bn.size, bncol[l]] = bn
        pk[0 : bp.size, bpcol[l]] = bp
    return {"wpack": pk}


_NC_CACHE = None


def kernel(**inputs) -> np.ndarray:
    global _NC_CACHE
    if _NC_CACHE is None:
        _NC_CACHE = _build_program()
    nc = _NC_CACHE

    x = np.asarray(inputs["input"], dtype=np.float32)
    assert x.shape == (N_TOTAL, 2)
    shared = _transform_weights(inputs)

    in_maps = []
    for c in range(N_CORES):
        m = dict(shared)
        m["xt"] = np.ascontiguousarray(x[c * P : (c + 1) * P].T)
        in_maps.append(m)

    res = bass_utils.run_bass_kernel_spmd(nc, in_maps, core_ids=list(range(N_CORES)))
    b8 = np.float32(np.asarray(inputs["b8"], dtype=np.float32).reshape(()))
    y = np.concatenate([res.results[c]["y"][0] for c in range(N_CORES)])
    y = (y + b8).astype(np.float32)
    return y.reshape(N_TOTAL, 1)


# revision 40
# speedup vs baseline: 1.9428x; 1.0010x over previous
"""Trainium2 Bass kernel for nn_Decoder (NeRF-style 9-layer MLP, Softplus(beta=100)).

Strategy (pure data parallel over 8 cores, feature-major layout):
  - activations live in SBUF as z_l = 100 * y_l (scaled softplus outputs), shape
    [features<=100 partitions, points free-dim]; weights are stationary lhsT.
  - matmuls run in float32r (1 cycle/row for free-dim >= 256, 4x faster than
    plain fp32; numerically exact in fp32 accumulation).
  - per layer, per supertile [100, T]:
      PE : psum = W z                        (T/512 matmuls, N=512 each)
      ACT: s = Sigmoid(-psum - 100b)         (= sigma(-u), one table op)
      DVE: z' = max(u + g, g)                (one fused custom op)
            where u = psum + 100b,  p = min(s, 1-s),
                  g = (c2*p + c1)*p  ~=  -ln(1-p)  (minimax on [0, 1/2])
    Math: softplus(u) = relu(u) + (-ln(1 - min(sigma(-u), sigma(u)))).
    The deg-2 minimax of -ln(1-p) has max err 5.4e-3 in z units (5.4e-5 in y),
    end-to-end rel err ~7e-4. Exact in both tails (p->0 as |u|->inf).
  - skip connection (layer 4) handled by DMAing the raw input into partitions
    98:100 of the layer-3 output tile; layer-4 weight columns scaled to match.
  - layer 8 (100->1, no activation): matmul into psum row 0, DMA straight to
    DRAM; the scalar bias b8 is added on the host after the gather.
Supertiles are emitted software-pipelined in groups of GRP=4 (layers
interleaved across the group) so each engine's in-order stream never
head-of-line blocks on the serial MM->ACT->DVE chain of a single supertile.
"""

import numpy as np

import concourse.bass as bass
import concourse.tile as tile
from concourse import bacc, mybir
from concourse import bass_utils
from concourse.bass_interp import get_hw_module

F32 = mybir.dt.float32
F32R = mybir.dt.float32r
ACTF = mybir.ActivationFunctionType

N_CORES = 8
N_TOTAL = 1048576
P = N_TOTAL // N_CORES          # 131072 points per core
DIMS = [2, 100, 100, 100, 98, 100, 100, 100, 100, 1]

# deg-2 minimax of -ln(1-p) on [0, 0.5]: g = (SP_C2*p + SP_C1)*p
SP_C1 = 0.94427875
SP_C2 = 0.86235463

_SOFTPLUS_SIG = None


def _wpack_layout():
    """Column layout of the packed weight/bias tensor [100, WPK].  Layer 0's
    weights and biases occupy the first columns so a slim head DMA can launch
    the pipeline while the rest streams in."""
    wcol = [0] * 8
    bncol = [0] * 8
    bpcol = [0] * 8
    c = DIMS[1]          # lhsT0
    bncol[0] = c
    bpcol[0] = c + 1
    c += 2
    head = c             # end of the slim head
    for l in range(1, 8):
        wcol[l] = c
        c += DIMS[l + 1]
    w8col = []
    for i in range(16):
        w8col.append(c)
        c += 16
    for l in range(1, 8):
        bncol[l] = c
        bpcol[l] = c + 1
        c += 2
    return wcol, w8col, bncol, bpcol, head, c


def _get_softplus_sig():
    """Register (once) the fused custom-DVE op computing, per element,
        z = max(u + g, g)   with u = in1 + s0,  p = min(in0, 1-in0),
                                 g = (imm2*p + s1)*p
    in0 = sigma(-u) tile (SBUF), in1 = psum (PSUM), s0 = +100*b [P,1],
    s1 = SP_C1, imm2 = SP_C2.  Exactly 8 ALU stages (v3 pipeline depth)."""
    global _SOFTPLUS_SIG
    if _SOFTPLUS_SIG is not None:
        return _SOFTPLUS_SIG
    from concourse import dve_ops
    from concourse.dve_spec import (
        Spec, Src0, Src1, C0, C1, C2, One, lower, maxx, minn, _has_src1,
    )
    from concourse.dve_uop import DveOpSpec

    name = "SOFTPLUS_SIG_ANT"
    _p = minn(Src0, One - Src0)
    _u = Src1 + C0
    _g = (C2 * _p + C1) * _p
    body = maxx(_u + _g, _g)

    def _ref(in0, in1, s0, s1, imm2):
        s = np.asarray(in0, dtype=np.float32)
        u = (np.asarray(in1, dtype=np.float32) + np.asarray(s0, np.float32)).astype(
            np.float32
        )
        w = (np.float32(1.0) - s).astype(np.float32)
        p = np.minimum(s, w)
        g = (
            (np.float32(imm2) * p + np.asarray(s1, np.float32)).astype(np.float32) * p
        ).astype(np.float32)
        return np.maximum((u + g).astype(np.float32), g)

    spec = Spec(body=body, reference=_ref)
    op = dve_ops.DveOp(name, spec, subdim=False, uops_sha={})
    dve_ops.OPS.append(op)
    dve_ops.CUSTOM_DVE_SPECS[name] = spec
    dve_ops._SUB_OPCODE_FOR_NAME[name] = (
        dve_ops._CUSTOM_DVE_ROW_BASE + len(dve_ops.OPS) - 1
    )
    assert dve_ops._SUB_OPCODE_FOR_NAME[name] < 0x20
    for ver in ("v3", "v4"):
        uops = lower(spec, ver=ver)
        tmp = DveOpSpec(
            name=name,
            opcode=dve_ops.get_dve_sub_opcode(name),
            uops=uops,
            rd1_en=_has_src1(spec),
        )
        op.uops_sha[ver] = tmp.sha(ver)
    _SOFTPLUS_SIG = op
    return _SOFTPLUS_SIG


def _build_program(T=1024, psum_bufs=4, sbufs=8, mbufs=16, GRP=4, xbufs=6):
    NT = P // T
    sp_sig = _get_softplus_sig()
    nc = bacc.Bacc(
        "TRN2",
        target_bir_lowering=False,
        debug=False,
        enable_asserts=False,
        num_devices=N_CORES,
    )

    # DRAM I/O (per core).  All weights/biases arrive in ONE packed [100, WPK]
    # tensor (one DMA instead of ~30 -- the serial HWDGE preamble was 20us):
    # columns [wcol[l] : wcol[l]+out_dim] hold lhsT_l (valid on partitions
    # 0:in_dim), then 4 one-hot layer-8 weight blocks of 4 columns each, then
    # 8 columns of bneg and 8 of bpos (valid on partitions 0:out_dim).
    wcol, w8col, bncol, bpcol, WHEAD, WPK = _wpack_layout()
    xt_d = nc.dram_tensor("xt", [2, P], F32, kind="ExternalInput")
    wpk_d = nc.dram_tensor("wpack", [100, WPK], F32, kind="ExternalInput")
    y_d = nc.dram_tensor("y", [1, P], F32, kind="ExternalOutput")

    with tile.TileContext(nc) as tc:
        with (
            tc.tile_pool(name="wpool", bufs=1) as wpool,
            tc.tile_pool(name="xpool", bufs=xbufs) as xpool,
            tc.tile_pool(name="psum", bufs=psum_bufs, space="PSUM") as pspool,
            tc.tile_pool(name="spool", bufs=sbufs) as spool,
            tc.tile_pool(name="mpool", bufs=mbufs) as mpool,
            tc.tile_pool(name="opool", bufs=4) as opool,
        ):
            # dummy 1-element sigmoid with no upstream deps: walrus places
            # the ACT table load before it, so the load runs at t~0 instead
            # of gating the first real sigmoid
            dz = wpool.tile([1, 1], F32, tag="dz")
            nc.vector.memset(dz[:], 0.0)
            dzo = wpool.tile([1, 1], F32, tag="dzo")
            nc.scalar.activation(dzo[:], dz[:], ACTF.Sigmoid, bias=0.0, scale=1.0)
            # --- first input tile, then weights: slim head (layer-0 weights
            # and biases) first so the first matmul chain launches early
            xt0 = xpool.tile([2, T], F32R, tag="xt")
            nc.sync.dma_start(xt0[:], xt_d.ap()[:, bass.ts(0, T)].bitcast(F32R))
            wpk = wpool.tile([100, WPK], F32R, tag="wpack")
            # slim head on the ACT HWDGE queue: overlaps xt0 on the SP queue
            # (the table load was already hoisted ahead of it by the dummy)
            nc.scalar.dma_start(wpk[0:100, 0:WHEAD], wpk_d.ap()[:, 0:WHEAD].bitcast(F32R))
            nc.sync.dma_start(wpk[0:100, WHEAD:WPK], wpk_d.ap()[:, WHEAD:WPK].bitcast(F32R))
            wts = []
            for l in range(8):
                in_dim = 100 if l == 4 else DIMS[l]
                out_dim = DIMS[l + 1]
                wts.append(wpk[0:in_dim, wcol[l] : wcol[l] + out_dim])
            w8ts = [wpk[0:100, w8col[i] : w8col[i] + 16] for i in range(16)]
            bnegs = [
                wpk[0 : DIMS[l + 1], bncol[l] : bncol[l] + 1].bitcast(F32)
                for l in range(8)
            ]
            bposs = [
                wpk[0 : DIMS[l + 1], bpcol[l] : bpcol[l] + 1].bitcast(F32)
                for l in range(8)
            ]

            # --- main loop ---
            assert NT % GRP == 0 and GRP == 4

            def emit_l8(col0, prevs):
                # accumulate the GRP supertiles' scalar outputs into a single
                # [16, 256] psum tile (row 4i+j = supertile i, quarter j) via
                # one-hot weight columns; the ACT eviction copy then has free
                # size 256 so it barely perturbs the sigma stream.
                nq = T // 256
                ps = pspool.tile([100, T], F32, tag="ps")
                k = 0
                for i in range(GRP):
                    for j in range(nq):
                        js = bass.ts(j, 256)
                        nc.tensor.matmul(
                            ps[0:16, 0:256], w8ts[k], prevs[i][0:100, js],
                            start=(k == 0), stop=(k == GRP * nq - 1),
                        )
                        k += 1
                out_t = opool.tile([16, 256], F32, tag="out")
                nc.scalar.copy(out_t[:], ps[0:16, 0:256])
                # one DMA for the whole group: row 4i+j of out_t lands at
                # y[col0 + 1024*i + 256*j : +256] -- rows map contiguously
                nc.sync.dma_start(
                    y_d.ap()[:, col0 : col0 + GRP * T], out_t[0:16, :]
                )

            deferred = None
            for g in range(NT // GRP):
                ts_ids = [g * GRP + i for i in range(GRP)]
                sls = [bass.ts(t, T) for t in ts_ids]
                xts = []
                for i, sl in enumerate(sls):
                    if g == 0 and i == 0:
                        xts.append(xt0)
                        continue
                    xt = xpool.tile([2, T], F32R, tag="xt")
                    nc.sync.dma_start(xt[:], xt_d.ap()[:, sl].bitcast(F32R))
                    xts.append(xt)
                prevs = list(xts)
                for l in range(8):
                    if l == DEFER and deferred is not None:
                        emit_l8(*deferred)
                        deferred = None
                    in_dim = 100 if l == 4 else DIMS[l]
                    out_dim = DIMS[l + 1]
                    pss = []
                    for i in range(GRP):
                        ps = pspool.tile([100, T], F32, tag="ps")
                        for j in range(T // 512):
                            js = bass.ts(j, 512)
                            rhs = prevs[i][0:in_dim, js]
                            nc.tensor.matmul(
                                ps[0:out_dim, js],
                                wts[l],
                                rhs,
                                start=True,
                                stop=True,
                            )
                        pss.append(ps)
                    sts = []
                    for i in range(GRP):
                        st = spool.tile([100, T], F32, tag="s")
                        nc.scalar.activation(
                            st[0:out_dim, :], pss[i][0:out_dim, :], ACTF.Sigmoid,
                            bias=bnegs[l], scale=-1.0,
                        )
                        sts.append(st)
                    nprevs = []
                    for i in range(GRP):
                        m = mpool.tile([100, T], F32R, tag="m7" if l == 7 else "m")
                        out_ap = m[0:out_dim, :]
                        if l == 3:
                            nc.sync.dma_start(
                                m[98:100, :], xt_d.ap()[:, sls[i]].bitcast(F32R)
                            )
                        nc.vector._custom_dve(
                            sp_sig,
                            out=out_ap,
                            in0=sts[i][0:out_dim, :],
                            in1=pss[i][0:out_dim, :],
                            s0=bposs[l],
                            s1=SP_C1,
                            imm2=SP_C2,
                        )
                        nprevs.append(m)
                    prevs = nprevs
                if deferred is not None:
                    emit_l8(*deferred)
                deferred = (ts_ids[0] * T, prevs)
            if deferred is not None:
                emit_l8(*deferred)

    nc.compile()
    nc.m = get_hw_module(nc.m)
    return nc


def _transform_weights(inputs):
    """Host-side weight/bias transform -> one packed DRAM tensor (shared
    across cores)."""
    W = [np.asarray(inputs[f"W{l}"], dtype=np.float32) for l in range(9)]
    b = [np.asarray(inputs[f"b{l}"], dtype=np.float32) for l in range(9)]
    wcol, w8col, bncol, bpcol, WHEAD, WPK = _wpack_layout()
    pk = np.zeros((100, WPK), dtype=np.float32)
    lhsT = {}
    lhsT[0] = (100.0 * W[0]).T.astype(np.float32)
    for l in (1, 2, 3, 5, 6, 7):
        lhsT[l] = W[l].T
    # layer-4 input tile layout: partitions 0:98 = z3, 98:100 = raw x
    lhsT[4] = np.concatenate(
        [W[4][:, 2:].T, (100.0 * W[4][:, :2]).T.astype(np.float32)], axis=0
    )
    for l in range(8):
        t = lhsT[l]
        pk[0 : t.shape[0], wcol[l] : wcol[l] + t.shape[1]] = t
    for i in range(16):
        pk[:, w8col[i] + i] = W[8].reshape(-1) / 100.0
    for l in range(8):
        bn = (-100.0 * b[l]).astype(np.float32)
        bp = (100.0 * b[l]).astype(np.float32)
        pk[0 : bn.size, bncol[l]] = bn
        pk[0 : bp.size, bpcol[l]] = bp
    return {"wpack": pk}


_NC_CACHE = None


def kernel(**inputs) -> np.ndarray:
    global _NC_CACHE
    if _NC_CACHE is None:
        _NC_CACHE = _build_program()
    nc = _NC_CACHE

    x = np.asarray(inputs["input"], dtype=np.float32)
    assert x.shape == (N_TOTAL, 2)
    shared = _transform_weights(inputs)

    in_maps = []
    for c in range(N_CORES):
        m = dict(shared)
        m["xt"] = np.ascontiguousarray(x[c * P : (c + 1) * P].T)
        in_maps.append(m)

    res = bass_utils.run_bass_kernel_spmd(nc, in_maps, core_ids=list(range(N_CORES)))
    b8 = np.float32(np.asarray(inputs["b8"], dtype=np.float32).reshape(()))
    y = np.concatenate([res.results[c]["y"][0] for c in range(N_CORES)])
    y = (y + b8).astype(np.float32)
    return y.reshape(N_TOTAL, 1)


# revision 43
# speedup vs baseline: 1.9429x; 1.0000x over previous
"""Trainium2 Bass kernel for nn_Decoder (NeRF-style 9-layer MLP, Softplus(beta=100)).

Strategy (pure data parallel over 8 cores, feature-major layout):
  - activations live in SBUF as z_l = 100 * y_l (scaled softplus outputs), shape
    [features<=100 partitions, points free-dim]; weights are stationary lhsT.
  - matmuls run in float32r (1 cycle/row for free-dim >= 256, 4x faster than
    plain fp32; numerically exact in fp32 accumulation).
  - per layer, per supertile [100, T]:
      PE : psum = W z                        (T/512 matmuls, N=512 each)
      ACT: s = Sigmoid(-psum - 100b)         (= sigma(-u), one table op)
      DVE: z' = max(u + g, g)                (one fused custom op)
            where u = psum + 100b,  p = min(s, 1-s),
                  g = (c2*p + c1)*p  ~=  -ln(1-p)  (minimax on [0, 1/2])
    Math: softplus(u) = relu(u) + (-ln(1 - min(sigma(-u), sigma(u)))).
    The deg-2 minimax of -ln(1-p) has max err 5.4e-3 in z units (5.4e-5 in y),
    end-to-end rel err ~7e-4. Exact in both tails (p->0 as |u|->inf).
  - skip connection (layer 4) handled by DMAing the raw input into partitions
    98:100 of the layer-3 output tile; layer-4 weight columns scaled to match.
  - layer 8 (100->1, no activation): matmul into psum row 0, DMA straight to
    DRAM; the scalar bias b8 is added on the host after the gather.
Supertiles are emitted software-pipelined in groups of GRP=4 (layers
interleaved across the group) so each engine's in-order stream never
head-of-line blocks on the serial MM->ACT->DVE chain of a single supertile.
"""

import numpy as np

import concourse.bass as bass
import concourse.tile as tile
from concourse import bacc, mybir
from concourse import bass_utils
from concourse.bass_interp import get_hw_module

F32 = mybir.dt.float32
F32R = mybir.dt.float32r
ACTF = mybir.ActivationFunctionType

N_CORES = 8
N_TOTAL = 1048576
P = N_TOTAL // N_CORES          # 131072 points per core
DIMS = [2, 100, 100, 100, 98, 100, 100, 100, 100, 1]

# deg-2 minimax of -ln(1-p) on [0, 0.5]: g = (SP_C2*p + SP_C1)*p
SP_C1 = 0.94427875
SP_C2 = 0.86235463

_SOFTPLUS_SIG = None


def _wpack_layout():
    """Column layout of the packed weight/bias tensor [100, WPK].  Layer 0's
    weights and biases occupy the first columns so a slim head DMA can launch
    the pipeline while the rest streams in."""
    wcol = [0] * 8
    bncol = [0] * 8
    bpcol = [0] * 8
    c = DIMS[1]          # lhsT0
    bncol[0] = c
    bpcol[0] = c + 1
    c += 2
    head = c             # end of the slim head
    for l in range(1, 8):
        wcol[l] = c
        c += DIMS[l + 1]
    w8col = []
    for i in range(16):
        w8col.append(c)
        c += 16
    for l in range(1, 8):
        bncol[l] = c
        bpcol[l] = c + 1
        c += 2
    return wcol, w8col, bncol, bpcol, head, c


def _get_softplus_sig():
    """Register (once) the fused custom-DVE op computing, per element,
        z = max(u + g, g)   with u = in1 + s0,  p = min(in0, 1-in0),
                                 g = (imm2*p + s1)*p
    in0 = sigma(-u) tile (SBUF), in1 = psum (PSUM), s0 = +100*b [P,1],
    s1 = SP_C1, imm2 = SP_C2.  Exactly 8 ALU stages (v3 pipeline depth)."""
    global _SOFTPLUS_SIG
    if _SOFTPLUS_SIG is not None:
        return _SOFTPLUS_SIG
    from concourse import dve_ops
    from concourse.dve_spec import (
        Spec, Src0, Src1, C0, C1, C2, One, lower, maxx, minn, _has_src1,
    )
    from concourse.dve_uop import DveOpSpec

    name = "SOFTPLUS_SIG_ANT"
    _p = minn(Src0, One - Src0)
    _u = Src1 + C0
    _g = (C2 * _p + C1) * _p
    body = maxx(_u + _g, _g)

    def _ref(in0, in1, s0, s1, imm2):
        s = np.asarray(in0, dtype=np.float32)
        u = (np.asarray(in1, dtype=np.float32) + np.asarray(s0, np.float32)).astype(
            np.float32
        )
        w = (np.float32(1.0) - s).astype(np.float32)
        p = np.minimum(s, w)
        g = (
            (np.float32(imm2) * p + np.asarray(s1, np.float32)).astype(np.float32) * p
        ).astype(np.float32)
        return np.maximum((u + g).astype(np.float32), g)

    spec = Spec(body=body, reference=_ref)
    op = dve_ops.DveOp(name, spec, subdim=False, uops_sha={})
    dve_ops.OPS.append(op)
    dve_ops.CUSTOM_DVE_SPECS[name] = spec
    dve_ops._SUB_OPCODE_FOR_NAME[name] = (
        dve_ops._CUSTOM_DVE_ROW_BASE + len(dve_ops.OPS) - 1
    )
    assert dve_ops._SUB_OPCODE_FOR_NAME[name] < 0x20
    for ver in ("v3", "v4"):
        uops = lower(spec, ver=ver)
        tmp = DveOpSpec(
            name=name,
            opcode=dve_ops.get_dve_sub_opcode(name),
            uops=uops,
            rd1_en=_has_src1(spec),
        )
        op.uops_sha[ver] = tmp.sha(ver)
    _SOFTPLUS_SIG = op
    return _SOFTPLUS_SIG


def _build_program(T=1024, psum_bufs=4, sbufs=8, mbufs=16, GRP=4, xbufs=6):
    NT = P // T
    sp_sig = _get_softplus_sig()
    nc = bacc.Bacc(
        "TRN2",
        target_bir_lowering=False,
        debug=False,
        enable_asserts=False,
        num_devices=N_CORES,
    )

    # DRAM I/O (per core).  All weights/biases arrive in ONE packed [100, WPK]
    # tensor (one DMA instead of ~30 -- the serial HWDGE preamble was 20us):
    # columns [wcol[l] : wcol[l]+out_dim] hold lhsT_l (valid on partitions
    # 0:in_dim), then 4 one-hot layer-8 weight blocks of 4 columns each, then
    # 8 columns of bneg and 8 of bpos (valid on partitions 0:out_dim).
    wcol, w8col, bncol, bpcol, WHEAD, WPK = _wpack_layout()
    xt_d = nc.dram_tensor("xt", [2, P], F32, kind="ExternalInput")
    wpk_d = nc.dram_tensor("wpack", [100, WPK], F32, kind="ExternalInput")
    y_d = nc.dram_tensor("y", [1, P], F32, kind="ExternalOutput")

    with tile.TileContext(nc) as tc:
        with (
            tc.tile_pool(name="wpool", bufs=1) as wpool,
            tc.tile_pool(name="xpool", bufs=xbufs) as xpool,
            tc.tile_pool(name="psum", bufs=psum_bufs, space="PSUM") as pspool,
            tc.tile_pool(name="spool", bufs=sbufs) as spool,
            tc.tile_pool(name="mpool", bufs=mbufs) as mpool,
            tc.tile_pool(name="opool", bufs=4) as opool,
        ):
            # dummy 1-element sigmoid with no upstream deps: walrus places
            # the ACT table load before it, so the load runs at t~0 instead
            # of gating the first real sigmoid
            dz = wpool.tile([1, 1], F32, tag="dz")
            nc.vector.memset(dz[:], 0.0)
            dzo = wpool.tile([1, 1], F32, tag="dzo")
            nc.scalar.activation(dzo[:], dz[:], ACTF.Sigmoid, bias=0.0, scale=1.0)
            # dummy matmuls warm the PE p-state while the input/weight DMAs
            # are in flight, so the first real matmuls run at mid/full clock
            dmm = wpool.tile([1, 512], F32R, tag="dmm")
            nc.vector.memset(dmm[:].bitcast(F32), 0.0)
            dps = pspool.tile([100, T], F32, tag="ps")
            for _ in range(4):
                nc.tensor.matmul(
                    dps[0:1, 0:512], dmm[0:1, 0:1], dmm[0:1, 0:512],
                    start=True, stop=True,
                )
            # --- first input tile, then weights: slim head (layer-0 weights
            # and biases) first so the first matmul chain launches early
            xt0 = xpool.tile([2, T], F32R, tag="xt")
            nc.sync.dma_start(xt0[:], xt_d.ap()[:, bass.ts(0, T)].bitcast(F32R))
            wpk = wpool.tile([100, WPK], F32R, tag="wpack")
            # slim head on the ACT HWDGE queue: overlaps xt0 on the SP queue
            # (the table load was already hoisted ahead of it by the dummy)
            nc.scalar.dma_start(wpk[0:100, 0:WHEAD], wpk_d.ap()[:, 0:WHEAD].bitcast(F32R))
            nc.sync.dma_start(wpk[0:100, WHEAD:WPK], wpk_d.ap()[:, WHEAD:WPK].bitcast(F32R))
            wts = []
            for l in range(8):
                in_dim = 100 if l == 4 else DIMS[l]
                out_dim = DIMS[l + 1]
                wts.append(wpk[0:in_dim, wcol[l] : wcol[l] + out_dim])
            w8ts = [wpk[0:100, w8col[i] : w8col[i] + 16] for i in range(16)]
            bnegs = [
                wpk[0 : DIMS[l + 1], bncol[l] : bncol[l] + 1].bitcast(F32)
                for l in range(8)
            ]
            bposs = [
                wpk[0 : DIMS[l + 1], bpcol[l] : bpcol[l] + 1].bitcast(F32)
                for l in range(8)
            ]

            # --- main loop ---
            assert NT % GRP == 0 and GRP == 4

            def emit_l8(col0, prevs, final=False):
                # accumulate the GRP supertiles' scalar outputs into a single
                # [16, 256] psum tile (row 4i+j = supertile i, quarter j) via
                # one-hot weight columns; the ACT eviction copy then has free
                # size 256 so it barely perturbs the sigma stream.
                nq = T // 256
                ps = pspool.tile([100, T], F32, tag="ps")
                k = 0
                for i in range(GRP):
                    for j in range(nq):
                        js = bass.ts(j, 256)
                        nc.tensor.matmul(
                            ps[0:16, 0:256], w8ts[k], prevs[i][0:100, js],
                            start=(k == 0), stop=(k == GRP * nq - 1),
                        )
                        k += 1
                out_t = opool.tile([16, 256], F32, tag="out")
                nc.scalar.copy(out_t[:], ps[0:16, 0:256])
                # one DMA for the whole group: row 4i+j of out_t lands at
                # y[col0 + 1024*i + 256*j : +256] -- rows map contiguously
                nc.sync.dma_start(
                    y_d.ap()[:, col0 : col0 + GRP * T], out_t[0:16, :]
                )

            deferred = None
            for g in range(NT // GRP):
                ts_ids = [g * GRP + i for i in range(GRP)]
                sls = [bass.ts(t, T) for t in ts_ids]
                xts = []
                for i, sl in enumerate(sls):
                    if g == 0 and i == 0:
                        xts.append(xt0)
                        continue
                    xt = xpool.tile([2, T], F32R, tag="xt")
                    nc.sync.dma_start(xt[:], xt_d.ap()[:, sl].bitcast(F32R))
                    xts.append(xt)
                prevs = list(xts)
                for l in range(8):
                    if l == DEFER and deferred is not None:
                        emit_l8(*deferred)
                        deferred = None
                    in_dim = 100 if l == 4 else DIMS[l]
                    out_dim = DIMS[l + 1]
                    pss = []
                    for i in range(GRP):
                        ps = pspool.tile([100, T], F32, tag="ps")
                        for j in range(T // 512):
                            js = bass.ts(j, 512)
                            rhs = prevs[i][0:in_dim, js]
                            nc.tensor.matmul(
                                ps[0:out_dim, js],
                                wts[l],
                                rhs,
                                start=True,
                                stop=True,
                            )
                        pss.append(ps)
                    sts = []
                    for i in range(GRP):
                        st = spool.tile([100, T], F32, tag="s")
                        nc.scalar.activation(
                            st[0:out_dim, :], pss[i][0:out_dim, :], ACTF.Sigmoid,
                            bias=bnegs[l], scale=-1.0,
                        )
                        sts.append(st)
                    nprevs = []
                    for i in range(GRP):
                        m = mpool.tile([100, T], F32R, tag="m7" if l == 7 else "m")
                        out_ap = m[0:out_dim, :]
                        if l == 3:
                            nc.sync.dma_start(
                                m[98:100, :], xt_d.ap()[:, sls[i]].bitcast(F32R)
                            )
                        nc.vector._custom_dve(
                            sp_sig,
                            out=out_ap,
                            in0=sts[i][0:out_dim, :],
                            in1=pss[i][0:out_dim, :],
                            s0=bposs[l],
                            s1=SP_C1,
                            imm2=SP_C2,
                        )
                        nprevs.append(m)
                    prevs = nprevs
                if deferred is not None:
                    emit_l8(*deferred)
                deferred = (ts_ids[0] * T, prevs)
            if deferred is not None:
                emit_l8(*deferred, final=True)

    nc.compile()
    nc.m = get_hw_module(nc.m)
    return nc


def _transform_weights(inputs):
    """Host-side weight/bias transform -> one packed DRAM tensor (shared
    across cores)."""
    W = [np.asarray(inputs[f"W{l}"], dtype=np.float32) for l in range(9)]
    b = [np.asarray(inputs[f"b{l}"], dtype=np.float32) for l in range(9)]
    wcol, w8col, bncol, bpcol, WHEAD, WPK = _wpack_layout()
    pk = np.zeros((100, WPK), dtype=np.float32)
    lhsT = {}
    lhsT[0] = (100.0 * W[0]).T.astype(np.float32)
    for l in (1, 2, 3, 5, 6, 7):
        lhsT[l] = W[l].T
    # layer-4 input tile layout: partitions 0:98 = z3, 98:100 = raw x
    lhsT[4] = np.concatenate(
        [W[4][:, 2:].T, (100.0 * W[4][:, :2]).T.astype(np.float32)], axis=0
    )
    for l in range(8):
        t = lhsT[l]
        pk[0 : t.shape[0], wcol[l] : wcol[l] + t.shape[1]] = t
    for i in range(16):
        pk[:, w8col[i] + i] = W[8].reshape(-1) / 100.0
    for l in range(8):
        bn = (-100.0 * b[l]).astype(np.float32)
        bp = (100.0 * b[l]).astype(np.float32)
        pk[0 : bn.size, bncol[l]] = bn
        pk[0 : bp.size, bpcol[l]] = bp
    return {"wpack": pk}


_NC_CACHE = None


def kernel(**inputs) -> np.ndarray:
    global _NC_CACHE
    if _NC_CACHE is None:
        _NC_CACHE = _build_program()
    nc = _NC_CACHE

    x = np.asarray(inputs["input"], dtype=np.float32)
    assert x.shape == (N_TOTAL, 2)
    shared = _transform_weights(inputs)

    in_maps = []
    for c in range(N_CORES):
        m = dict(shared)
        m["xt"] = np.ascontiguousarray(x[c * P : (c + 1) * P].T)
        in_maps.append(m)

    res = bass_utils.run_bass_kernel_spmd(nc, in_maps, core_ids=list(range(N_CORES)))
    b8 = np.float32(np.asarray(inputs["b8"], dtype=np.float32).reshape(()))
    y = np.concatenate([res.results[c]["y"][0] for c in range(N_CORES)])
    y = (y + b8).astype(np.float32)
    return y.reshape(N_TOTAL, 1)
